# revision 5
# baseline (speedup 1.0000x reference)
"""GNN message-passing (AtomBondGraph) distributed Bass kernel for 8 TRN2 NeuronCores.

Engine-balanced: pair-batched relu (Act/DVE), de-interleaved scatter PSUM groups,
software-pipelined phase C with fused stats (accum_out), batched phase E.

Strategy: shard nodes (and their incoming edges) across 8 cores; h replicated as a
bf16 gather table, rebuilt each layer via AllGather. Edges are sorted by dst and
bucketed into 128-node windows (padded to a uniform tile count with zero-weight
edges) so segment_sum becomes PSUM-accumulated one-hot matmuls. BatchNorm batch
stats are AllReduced. MLP runs on transposed [feat, node] tiles so BN stats are
free-axis reductions and BN apply is a single fused scalar-engine activation.
"""
import numpy as np
import ml_dtypes
from contextlib import ExitStack

import concourse.bass as bass
from concourse import bacc
import concourse.tile as tile
from concourse import mybir
from concourse.bass_utils import run_bass_kernel_spmd
from concourse.masks import make_identity

bf16 = mybir.dt.bfloat16
f32 = mybir.dt.float32
i32 = mybir.dt.int32

NCORES = 8
V = 65536
D = 256
VSH = V // NCORES          # 8192 nodes per core
L = 5
WN = 128                   # nodes per window
NW = VSH // WN             # 64 windows per core
GW = 4                     # windows per group
NG = NW // GW              # 16 groups
CH = GW * WN               # 512 nodes per group
BN_EPS = 1e-5
AF = mybir.ActivationFunctionType
X_AXIS = mybir.AxisListType.X

_cache = {}
USE_DMAGATHER = True


def _build(TL, TH, epsl, dbg=False):
    Tw = TL + TH
    NT = NW * Tw           # tiles per core
    GT = GW * Tw           # tiles per group
    NLO = GW * TL * 128    # lo indices per group
    NHI = GW * TH * 128
    nc = bacc.Bacc(None, target_bir_lowering=False, debug=True)
    rg = [list(range(NCORES))]

    P_h0 = nc.declare_dram_parameter("h0", [V, D], bf16, isOutput=False)
    P_hT0 = nc.declare_dram_parameter("ht0", [128, 2 * VSH], bf16, isOutput=False)
    P_xsh = nc.declare_dram_parameter("xsh", [VSH, D], f32, isOutput=False)
    i16 = mybir.dt.int16
    P_idxlo = nc.declare_dram_parameter("idxlo", [NG, 128, NLO // 16], i16, isOutput=False)
    P_idxhi = nc.declare_dram_parameter("idxhi", [NG, 128, NHI // 16], i16, isOutput=False)
    P_oh48 = nc.declare_dram_parameter("oh48", [48, NT * 128], bf16, isOutput=False)
    P_dstoh = nc.declare_dram_parameter("dstoh", [128, NT * 128], bf16, isOutput=False)
    P_btab = nc.declare_dram_parameter("btab", [L, 48, D], bf16, isOutput=False)
    P_esl = nc.declare_dram_parameter("esl", [L, 128, 2], f32, isOutput=False)
    P_w1 = nc.declare_dram_parameter("w1", [L, 128, 1024], bf16, isOutput=False)
    P_w2 = nc.declare_dram_parameter("w2", [L, 128, 1024], bf16, isOutput=False)
    P_bn1 = nc.declare_dram_parameter("bn1", [L, 128, 8], f32, isOutput=False)
    P_bn2 = nc.declare_dram_parameter("bn2", [L, 128, 4], f32, isOutput=False)
    P_ieps = nc.declare_dram_parameter("ieps", [L, 128, 128], bf16, isOutput=False)
    P_out = nc.declare_dram_parameter("out", [VSH, D], f32, isOutput=True)
    P_dbg = nc.declare_dram_parameter("dbg", [16, 128, 2048], f32, isOutput=True) if dbg else None

    def dump(i, ap, width=2048):
        if P_dbg is not None:
            nc.gpsimd.dma_start(P_dbg[i, :, 0:width], ap)

    with tile.TileContext(nc) as tc, ExitStack() as ctx:
        const = ctx.enter_context(tc.tile_pool(name="const", bufs=1))
        wts = ctx.enter_context(tc.tile_pool(name="wts", bufs=2))
        big = ctx.enter_context(tc.tile_pool(name="big", bufs=1))
        st = ctx.enter_context(tc.tile_pool(name="st", bufs=2))
        sbp = ctx.enter_context(tc.tile_pool(name="sbp", bufs=2))
        msgp = ctx.enter_context(tc.tile_pool(name="msgp", bufs=4))
        ps = ctx.enter_context(tc.tile_pool(name="ps", bufs=3, space="PSUM"))
        psz = ctx.enter_context(tc.tile_pool(name="psz", bufs=1, space="PSUM"))
        psu = ctx.enter_context(tc.tile_pool(name="psu", bufs=2, space="PSUM"))
        psv = ctx.enter_context(tc.tile_pool(name="psv", bufs=2, space="PSUM"))
        dram = ctx.enter_context(tc.tile_pool(name="dram", bufs=2, space="DRAM"))

        ident = const.tile([128, 128], bf16)
        make_identity(nc, ident[:])
        zeros = const.tile([128, CH], bf16, tag="zeros")
        nc.vector.memset(zeros[:], 0.0)
        idx_lo = []
        idx_hi = []
        for g in range(NG):
            tl_ = const.tile([128, NLO // 16], mybir.dt.int16, tag=f"ixl{g}")
            nc.sync.dma_start(tl_[:], P_idxlo[g])
            idx_lo.append(tl_)
            th_ = const.tile([128, NHI // 16], mybir.dt.int16, tag=f"ixh{g}")
            nc.sync.dma_start(th_[:], P_idxhi[g])
            idx_hi.append(th_)
        hTown = big.tile([128, 2 * VSH], bf16)
        nc.sync.dma_start(hTown[:], P_hT0[:])
        zT = big.tile([128, 2 * VSH], bf16)
        vT = big.tile([128, 2 * VSH], bf16)

        hg_prev = None
        for l in range(L):
            w1sb = wts.tile([128, 1024], bf16, tag="w1")
            nc.sync.dma_start(w1sb[:], P_w1[l])
            w2sb = wts.tile([128, 1024], bf16, tag="w2")
            nc.sync.dma_start(w2sb[:], P_w2[l])
            btab = wts.tile([48, D], bf16, tag="bt")
            nc.sync.dma_start(btab[:], P_btab[l])
            eslsb = wts.tile([128, 2], f32, tag="esl")
            nc.sync.dma_start(eslsb[:], P_esl[l])
            iepsl = wts.tile([128, 128], bf16, tag="ie")
            nc.sync.dma_start(iepsl[:], P_ieps[l])
            bn1sb = wts.tile([128, 8], f32, tag="b1")
            nc.sync.dma_start(bn1sb[:], P_bn1[l])
            bn2sb = wts.tile([128, 4], f32, tag="b2")
            nc.sync.dma_start(bn2sb[:], P_bn2[l])

            st1 = st.tile([128, 8], f32, tag="st1")
            nc.gpsimd.memset(st1[:], 0)
            htab = P_h0 if l == 0 else hg_prev

            # ---- phase A: message passing + scatter + u-stats ----
            for g in range(NG):
                if USE_DMAGATHER:
                    gath_lo = sbp.tile([128, GW * TL * D], bf16, tag="gathlo")
                    nc.gpsimd.dma_gather(
                        out_ap=gath_lo[:].rearrange("p (s d) -> p s d", d=D),
                        in_ap=htab[0:V // 2, :],
                        idxs_ap=idx_lo[g][:], num_idxs=NLO, num_idxs_reg=NLO,
                        elem_size=D, single_packet=False)
                    gath_hi = sbp.tile([128, GW * TH * D], bf16, tag="gathhi")
                    nc.gpsimd.dma_gather(
                        out_ap=gath_hi[:].rearrange("p (s d) -> p s d", d=D),
                        in_ap=htab[V // 2:V, :],
                        idxs_ap=idx_hi[g][:], num_idxs=NHI, num_idxs_reg=NHI,
                        elem_size=D, single_packet=False)
                else:
                    gtiles = []
                    for tt in range(GT):
                        gt_ = sbp.tile([128, D], bf16, tag="gt", bufs=2 * GT + 2)
                        nc.gpsimd.indirect_dma_start(
                            out=gt_[:], out_offset=None, in_=htab[:],
                            in_offset=bass.IndirectOffsetOnAxis(
                                ap=idxf_sb[:, g * GT + tt:g * GT + tt + 1], axis=0))
                        gtiles.append(gt_)

                ohA = sbp.tile([48, GT * 128], bf16, tag="ohA")
                nc.sync.dma_start(ohA[:], P_oh48[:, g * GT * 128:(g + 1) * GT * 128])
                ohD = sbp.tile([128, GT * 128], bf16, tag="ohD")
                nc.sync.dma_start(ohD[:], P_dstoh[:, g * GT * 128:(g + 1) * GT * 128])
                for w in range(GW):
                    wg = g * GW + w
                    zps = psz.tile([128, 256], f32, tag="zps")
                    wtiles = [w * TL + j for j in range(TL)] + \
                             [GW * TL + w * TH + j for j in range(TH)]
                    nq = (Tw + 1) // 2
                    pairs2 = [wtiles[q * 2:(q + 1) * 2] for q in range(nq)]
                    msgs = []
                    for q, qt in enumerate(pairs2):
                        eps2 = ps.tile([128, 512], f32, tag="eps")
                        for j, ti in enumerate(qt):
                            nc.tensor.matmul(eps2[:, j * 256:(j + 1) * 256],
                                             lhsT=ohA[0:48, ti * 128:(ti + 1) * 128],
                                             rhs=btab[0:48, :], start=True, stop=False)
                            grhs = (gath_lo[:, ti * D:(ti + 1) * D] if ti < GW * TL
                                    else gath_hi[:, (ti - GW * TL) * D:(ti - GW * TL + 1) * D])
                            nc.tensor.matmul(eps2[:, j * 256:(j + 1) * 256],
                                             lhsT=ident[:], rhs=grhs,
                                             start=False, stop=True)
                        msg2 = msgp.tile([128, 512], bf16, tag="msg")
                        if q < 2:
                            nc.scalar.activation(msg2[:, 0:len(qt) * 256],
                                                 eps2[:, 0:len(qt) * 256], AF.Relu)
                        else:
                            nc.vector.tensor_scalar_max(msg2[:, 0:len(qt) * 256],
                                                        eps2[:, 0:len(qt) * 256], 0.0)
                        msgs.append(msg2)
                    for h in range(2):
                        first = True
                        for q, qt in enumerate(pairs2):
                            for j, ti in enumerate(qt):
                                last = (q == nq - 1) and (j == len(qt) - 1)
                                nc.tensor.matmul(
                                    zps[:, h * 128:(h + 1) * 128],
                                    lhsT=msgs[q][:, (j * 2 + h) * 128:(j * 2 + h + 1) * 128],
                                    rhs=ohD[:, ti * 128:(ti + 1) * 128],
                                    start=first, stop=last)
                                first = False
                    for h in range(2):
                        nc.vector.scalar_tensor_tensor(
                            out=zT[:, h * VSH + wg * 128: h * VSH + (wg + 1) * 128],
                            in0=hTown[:, h * VSH + wg * 128: h * VSH + (wg + 1) * 128],
                            scalar=float(1.0 + epsl[l]),
                            in1=zps[:, h * 128:(h + 1) * 128],
                            op0=mybir.AluOpType.mult, op1=mybir.AluOpType.add)
                # self-loop messages: relu(h + e_sl) added elementwise
                for k in range(2):
                    sl_ = slice(k * VSH + g * CH, k * VSH + (g + 1) * CH)
                    selfm = sbp.tile([128, CH], bf16, tag="sq")
                    nc.scalar.activation(selfm[:], hTown[:, sl_], AF.Relu,
                                         bias=eslsb[:, k:k + 1])
                    nc.gpsimd.tensor_add(zT[:, sl_], zT[:, sl_], selfm[:])
                # u-pass1: stats only (psv ring so phase C's psu allocs
                # have no WAR on this layer's Act stream during AR1)
                for m in range(4):
                    ups = psv.tile([128, CH], f32, tag="vps")
                    for k in range(2):
                        nc.tensor.matmul(
                            ups[:], lhsT=w1sb[:, k * 512 + m * 128: k * 512 + (m + 1) * 128],
                            rhs=zT[:, k * VSH + g * CH: k * VSH + (g + 1) * CH],
                            start=(k == 0), stop=(k == 1))
                    red = st.tile([128, 2], f32, tag="red")
                    nc.vector.reduce_sum(red[:, 0:1], ups[:], axis=X_AXIS)
                    sq = sbp.tile([128, CH], bf16, tag="sq")
                    nc.scalar.activation(sq[:], ups[:], AF.Square, accum_out=red[:, 1:2])
                    nc.vector.tensor_add(st1[:, m:m + 1], st1[:, m:m + 1], red[:, 0:1])
                    nc.vector.tensor_add(st1[:, 4 + m:5 + m], st1[:, 4 + m:5 + m], red[:, 1:2])

            if l == 0:
                dump(1, zT[:, 0:1024], 1024)
                dump(2, zT[:, VSH:VSH + 1024], 1024)

            # ---- AR1 + BN1 coefficients ----
            ar1i = dram.tile([128, 8], f32, tag="ar1i")
            ar1o = dram.tile([128, 8], f32, tag="ar1o", addr_space="Shared")
            nc.sync.dma_start(ar1i[:], st1[:])
            nc.gpsimd.collective_compute(
                "AllReduce", mybir.AluOpType.add, replica_groups=rg,
                ins=[ar1i.opt()], outs=[ar1o.opt()])
            st1g = st.tile([128, 8], f32, tag="st1g")
            nc.gpsimd.dma_start(st1g[:], ar1o[:])
            if l == 0:
                dump(3, st1g[:], 8)
            scb1 = st.tile([128, 8], f32, tag="scb1")
            tmp1 = st.tile([128, 12], f32, tag="tmp1")
            nc.vector.tensor_scalar_mul(tmp1[:, 0:4], st1g[:, 0:4], 1.0 / V)
            nc.vector.tensor_scalar_mul(tmp1[:, 4:8], st1g[:, 4:8], 1.0 / V)
            nc.vector.tensor_mul(tmp1[:, 8:12], tmp1[:, 0:4], tmp1[:, 0:4])
            nc.vector.tensor_sub(tmp1[:, 8:12], tmp1[:, 4:8], tmp1[:, 8:12])
            nc.vector.tensor_scalar_add(tmp1[:, 8:12], tmp1[:, 8:12], BN_EPS)
            nc.scalar.activation(scb1[:, 0:4], tmp1[:, 8:12], AF.Sqrt)
            nc.vector.reciprocal(scb1[:, 0:4], scb1[:, 0:4])
            nc.vector.tensor_mul(scb1[:, 0:4], scb1[:, 0:4], bn1sb[:, 0:4])
            nc.vector.tensor_mul(tmp1[:, 0:4], tmp1[:, 0:4], scb1[:, 0:4])
            nc.vector.tensor_sub(scb1[:, 4:8], bn1sb[:, 4:8], tmp1[:, 0:4])

            if l == 0:
                dump(4, scb1[:], 8)

            # ---- phase C: u-pass2 -> r -> v + v-stats (software-pipelined) ----
            st2 = st.tile([128, 4], f32, tag="st2")
            nc.gpsimd.memset(st2[:], 0)
            rTs = {}

            def c_u(g):
                rT = sbp.tile([128, 4 * CH], bf16, tag="rT")
                rTs[g] = rT
                for m in range(4):
                    ups = psu.tile([128, CH], f32, tag="ups")
                    for k in range(2):
                        nc.tensor.matmul(
                            ups[:], lhsT=w1sb[:, k * 512 + m * 128: k * 512 + (m + 1) * 128],
                            rhs=zT[:, k * VSH + g * CH: k * VSH + (g + 1) * CH],
                            start=(k == 0), stop=(k == 1))
                    nc.scalar.activation(rT[:, m * CH:(m + 1) * CH], ups[:], AF.Relu,
                                         scale=scb1[:, m:m + 1], bias=scb1[:, 4 + m:5 + m])

            def c_v(g):
                rT = rTs.pop(g)
                if l == 0 and g == 0:
                    dump(5, rT[:, 0:2048])
                for m2 in range(2):
                    vps = psv.tile([128, CH], f32, tag="vps")
                    for k in range(4):
                        nc.tensor.matmul(
                            vps[:], lhsT=w2sb[:, k * 256 + m2 * 128: k * 256 + (m2 + 1) * 128],
                            rhs=rT[:, k * CH:(k + 1) * CH],
                            start=(k == 0), stop=(k == 3))
                    red2 = st.tile([128, 2], f32, tag="red2")
                    sq2 = sbp.tile([128, CH], bf16, tag="sq")
                    nc.scalar.activation(sq2[:], vps[:], AF.Square, accum_out=red2[:, 1:2])
                    nc.vector.scalar_tensor_tensor(
                        out=vT[:, m2 * VSH + g * CH: m2 * VSH + (g + 1) * CH],
                        in0=vps[:], scalar=1.0, in1=zeros[:],
                        op0=mybir.AluOpType.mult, op1=mybir.AluOpType.add,
                        accum_out=red2[:, 0:1])
                    nc.vector.tensor_add(st2[:, m2:m2 + 1], st2[:, m2:m2 + 1], red2[:, 0:1])
                    nc.vector.tensor_add(st2[:, 2 + m2:3 + m2], st2[:, 2 + m2:3 + m2], red2[:, 1:2])

            c_u(0)
            for g in range(NG):
                if g + 1 < NG:
                    c_u(g + 1)
                c_v(g)

            # ---- AR2 + BN2 coefficients ----
            ar2i = dram.tile([128, 4], f32, tag="ar2i")
            ar2o = dram.tile([128, 4], f32, tag="ar2o", addr_space="Shared")
            nc.sync.dma_start(ar2i[:], st2[:])
            nc.gpsimd.collective_compute(
                "AllReduce", mybir.AluOpType.add, replica_groups=rg,
                ins=[ar2i.opt()], outs=[ar2o.opt()])
            st2g = st.tile([128, 4], f32, tag="st2g")
            nc.gpsimd.dma_start(st2g[:], ar2o[:])
            if l == 0:
                dump(6, st2g[:], 4)
            scb2 = st.tile([128, 4], f32, tag="scb2")
            tmp2 = st.tile([128, 6], f32, tag="tmp2")
            nc.vector.tensor_scalar_mul(tmp2[:, 0:2], st2g[:, 0:2], 1.0 / V)
            nc.vector.tensor_scalar_mul(tmp2[:, 2:4], st2g[:, 2:4], 1.0 / V)
            nc.vector.tensor_mul(tmp2[:, 4:6], tmp2[:, 0:2], tmp2[:, 0:2])
            nc.vector.tensor_sub(tmp2[:, 4:6], tmp2[:, 2:4], tmp2[:, 4:6])
            nc.vector.tensor_scalar_add(tmp2[:, 4:6], tmp2[:, 4:6], BN_EPS)
            nc.scalar.activation(scb2[:, 0:2], tmp2[:, 4:6], AF.Sqrt)
            nc.vector.reciprocal(scb2[:, 0:2], scb2[:, 0:2])
            nc.vector.tensor_mul(scb2[:, 0:2], scb2[:, 0:2], bn2sb[:, 0:2])
            nc.vector.tensor_mul(tmp2[:, 0:2], tmp2[:, 0:2], scb2[:, 0:2])
            nc.vector.tensor_sub(scb2[:, 2:4], bn2sb[:, 2:4], tmp2[:, 0:2])

            if l == 0:
                dump(7, scb2[:], 4)
                dump(8, vT[:, 0:1024], 1024)

            # ---- phase E: BN2 apply -> new h (transposed + row-major) ----
            agi = None
            if l < L - 1:
                agi = dram.tile([VSH, D], bf16, tag="agi")
            for g in range(NG):
                for m2 in range(2):
                    sl = slice(m2 * VSH + g * CH, m2 * VSH + (g + 1) * CH)
                    if l < L - 1:
                        nc.scalar.activation(
                            hTown[:, sl], vT[:, sl], AF.Relu,
                            scale=scb2[:, m2:m2 + 1], bias=scb2[:, 2 + m2:3 + m2])
                    else:
                        nc.vector.tensor_scalar(
                            out=hTown[:, sl], in0=vT[:, sl],
                            scalar1=scb2[:, m2:m2 + 1], scalar2=scb2[:, 2 + m2:3 + m2],
                            op0=mybir.AluOpType.mult, op1=mybir.AluOpType.add)
                for pair in range(2):
                    trp = ps.tile([128, 512], bf16, tag="eps")
                    for k in range(2):
                        wg = g * GW + pair * 2 + k
                        for m2 in range(2):
                            nc.tensor.transpose(
                                out=trp[:, (k * 2 + m2) * 128:(k * 2 + m2 + 1) * 128],
                                in_=hTown[:, m2 * VSH + wg * 128: m2 * VSH + (wg + 1) * 128],
                                identity=ident[:])
                    if l < L - 1:
                        hrow = sbp.tile([128, 2 * D], bf16, tag="hrow")
                        nc.vector.tensor_copy(hrow[:], trp[:])
                        for k in range(2):
                            wg = g * GW + pair * 2 + k
                            nc.sync.dma_start(agi[wg * 128:(wg + 1) * 128, :],
                                              hrow[:, k * 256:(k + 1) * 256])
                    else:
                        hrowf = sbp.tile([128, 2 * D], f32, tag="hrowf")
                        nc.vector.tensor_copy(hrowf[:], trp[:])
                        xw = sbp.tile([128, 2 * D], f32, tag="xw")
                        for k in range(2):
                            wg = g * GW + pair * 2 + k
                            nc.sync.dma_start(xw[:, k * 256:(k + 1) * 256],
                                              P_xsh[wg * 128:(wg + 1) * 128, :])
                        nc.vector.tensor_add(hrowf[:], hrowf[:], xw[:])
                        for k in range(2):
                            wg = g * GW + pair * 2 + k
                            nc.sync.dma_start(P_out[wg * 128:(wg + 1) * 128, :],
                                              hrowf[:, k * 256:(k + 1) * 256])

            dump(10 + l, hTown[:, 0:1024], 1024)
            if l < L - 1:
                hg = dram.tile([V, D], bf16, tag="hgat", addr_space="Shared")
                nc.gpsimd.collective_compute(
                    "AllGather", mybir.AluOpType.bypass, replica_groups=rg,
                    ins=[agi.opt()], outs=[hg.opt()])
                hg_prev = hg
    if not nc.is_finalized():
        nc.finalize()
    return nc


def _preprocess(x, edge_weight, bond_emb, W1, bn1_g, bn1_b, W2, bn2_g, bn2_b,
                eps_param, edge_index, edge_attr):
    to_bf = lambda a: np.asarray(a, np.float32).astype(ml_dtypes.bfloat16)
    ei = np.asarray(edge_index)
    # self-loops are handled by an elementwise relu(h + e_sl) path in the
    # kernel (their bond attr [5,7,0] is node-independent), so the edge
    # stream carries only the real edges
    src = ei[0].astype(np.int64)
    dst = ei[1].astype(np.int64)
    ea = np.asarray(edge_attr)
    ew = np.asarray(edge_weight, np.float32)

    # order edges by (window, src-half) so each window is [lo... , hi...]
    win = dst >> 7
    half = (src >= V // 2).astype(np.int64)
    order = np.lexsort((half, win))
    src, dst, ea, ew, half = src[order], dst[order], ea[order], ew[order], half[order]
    win = dst >> 7
    nwin = V // WN
    lo_cnt = np.bincount(win[half == 0], minlength=nwin)
    hi_cnt = np.bincount(win[half == 1], minlength=nwin)
    TL = max(1, int(np.ceil(lo_cnt.max() / 128)))
    TH = max(1, int(np.ceil(hi_cnt.max() / 128)))
    Tw = TL + TH

    # slot for each edge: windows laid out [nwin, Tw*128]; lo edges at
    # [0, lo_cnt), hi edges at [TL*128, TL*128 + hi_cnt)
    cnt = np.bincount(win, minlength=nwin)
    starts = np.zeros(nwin, np.int64)
    starts[1:] = np.cumsum(cnt)[:-1]
    pos_in_win = np.arange(len(dst)) - starts[win]     # 0..cnt-1, lo first
    pos = np.where(half == 0, pos_in_win, TL * 128 + (pos_in_win - lo_cnt[win]))
    slot = win * (Tw * 128) + pos
    NTOT = nwin * Tw * 128
    src_p = np.zeros(NTOT, np.int64); src_p[slot] = src
    ew_p = np.zeros(NTOT, np.float32); ew_p[slot] = ew
    dstl_p = np.zeros(NTOT, np.int32); dstl_p[slot] = dst & (WN - 1)
    ea_p = np.zeros((NTOT, 3), np.int32); ea_p[slot] = ea
    val_p = np.zeros(NTOT, np.float32); val_p[slot] = 1.0
    # hi pads must index the hi table: slot default src 0 is fine for lo pads;
    # hi pad slots get src V//2 (-> local 0)
    winslot = np.arange(NTOT) % (Tw * 128)
    is_hi_slot = winslot >= TL * 128
    pad = np.ones(NTOT, bool); pad[slot] = False
    src_p[pad & is_hi_slot] = V // 2

    # per-window slot arrays -> per-core tile arrays with group-region layout:
    # within a group of GW windows, tile t<GW*TL is (w=t//TL, lo j=t%TL);
    # tile t>=GW*TL is (w=(t-GW*TL)//TH, hi j=(t-GW*TL)%TH).
    NT = NW * Tw
    srcw = src_p.reshape(NCORES, NW, Tw, 128)
    eww = ew_p.reshape(NCORES, NW, Tw, 128)
    dstlw = dstl_p.reshape(NCORES, NW, Tw, 128)
    eaw = ea_p.reshape(NCORES, NW, Tw, 128, 3)
    valw = val_p.reshape(NCORES, NW, Tw, 128)

    def regroup(a):
        # [NW, Tw, 128, ...] -> [NG, GT, 128, ...] with group-region tile order
        sh = a.shape[3:]
        b = a.reshape(NG, GW, Tw, 128, *sh)
        lo = b[:, :, :TL]                      # [NG, GW, TL, 128, ...]
        hi = b[:, :, TL:]                      # [NG, GW, TH, 128, ...]
        lo = lo.reshape(NG, GW * TL, 128, *sh)
        hi = hi.reshape(NG, GW * TH, 128, *sh)
        return np.concatenate([lo, hi], axis=1)  # [NG, GT, 128, ...]

    h0 = np.asarray(x, np.float32)[:, 1:, :].reshape(V, D)
    h0b = h0.astype(ml_dtypes.bfloat16)
    hT0 = h0.reshape(NCORES, VSH, 2, 128).transpose(0, 3, 2, 1).reshape(
        NCORES, 128, 2 * VSH).astype(ml_dtypes.bfloat16)

    W1 = np.asarray(W1, np.float32)
    W2 = np.asarray(W2, np.float32)
    w1h = to_bf(W1.reshape(L, 2, 128, 512).transpose(0, 2, 1, 3).reshape(L, 128, 1024))
    w2h = to_bf(W2.reshape(L, 4, 128, 256).transpose(0, 2, 1, 3).reshape(L, 128, 1024))
    btabh = to_bf(np.asarray(bond_emb, np.float32).reshape(L, 48, D))
    be = np.asarray(bond_emb, np.float32)
    esl = be[:, 0, 5] + be[:, 1, 7] + be[:, 2, 0]          # [L, 256]
    eslT = np.ascontiguousarray(esl.reshape(L, 2, 128).transpose(0, 2, 1))
    g1 = np.asarray(bn1_g, np.float32).reshape(L, 4, 128).transpose(0, 2, 1)
    b1_ = np.asarray(bn1_b, np.float32).reshape(L, 4, 128).transpose(0, 2, 1)
    bn1h = np.concatenate([g1, b1_], -1).astype(np.float32)
    g2 = np.asarray(bn2_g, np.float32).reshape(L, 2, 128).transpose(0, 2, 1)
    b2_ = np.asarray(bn2_b, np.float32).reshape(L, 2, 128).transpose(0, 2, 1)
    bn2h = np.concatenate([g2, b2_], -1).astype(np.float32)
    epsv = np.asarray(eps_param, np.float32)
    iepsh = to_bf(np.eye(128, dtype=np.float32)[None] * (1.0 + epsv)[:, None, None])

    GT = GW * Tw
    NLO = GW * TL * 128
    NHI = GW * TH * 128
    t_idx = np.arange(NT)[:, None]
    p_idx = np.broadcast_to(np.arange(128)[None, :], (NT, 128))

    in_maps = []
    for c in range(NCORES):
        sg = regroup(srcw[c]).astype(np.int64)
        ewg = regroup(eww[c])
        dstlg = regroup(dstlw[c])
        eag = regroup(eaw[c])
        valg = regroup(valw[c])
        lo_l = sg[:, :GW * TL].reshape(NG, NLO)
        hi_l = sg[:, GW * TL:].reshape(NG, NHI) - V // 2
        hi_l = np.maximum(hi_l, 0)
        idxlo = np.ascontiguousarray(np.tile(
            lo_l.reshape(NG, NLO // 16, 16).transpose(0, 2, 1), (1, 8, 1))).astype(np.int16)
        idxhi = np.ascontiguousarray(np.tile(
            hi_l.reshape(NG, NHI // 16, 16).transpose(0, 2, 1), (1, 8, 1))).astype(np.int16)
        # flat [NT, 128] views for onehot construction
        ewf = ewg.reshape(NT, 128)
        dstlf = dstlg.reshape(NT, 128)
        eaf = eag.reshape(NT, 128, 3)
        valf = valg.reshape(NT, 128)
        dstoh = np.zeros((128, NT * 128), np.float32)
        dstoh[p_idx, t_idx * 128 + dstlf] = ewf
        oh48 = np.zeros((48, NT * 128), np.float32)
        colsE = t_idx * 128 + p_idx
        for f in range(3):
            oh48[f * 16 + eaf[..., f], colsE] = valf
        in_maps.append({
            "h0": h0b,
            "ht0": hT0[c],
            "xsh": h0[c * VSH:(c + 1) * VSH].copy(),
            "idxlo": idxlo,
            "idxhi": idxhi,
            "oh48": oh48.astype(ml_dtypes.bfloat16),
            "dstoh": dstoh.astype(ml_dtypes.bfloat16),
            "btab": btabh,
            "esl": eslT,
            "w1": w1h,
            "w2": w2h,
            "bn1": bn1h,
            "bn2": bn2h,
            "ieps": iepsh,
        })
    return TL, TH, in_maps


def kernel(x, edge_weight, bond_emb, W1, b1, bn1_g, bn1_b, W2, b2,
           bn2_g, bn2_b, eps_param, x_mask, edge_index, edge_attr,
           _trace=False, _dbg=False):
    # b1/b2 cancel inside the immediately-following BatchNorms; x_mask handled below.
    import time as _time
    _t0 = _time.time()
    in_maps = _preprocess(x, edge_weight, bond_emb, W1, bn1_g, bn1_b, W2,
                          bn2_g, bn2_b, eps_param, edge_index, edge_attr)
    print(f"[kernel] preprocess {_time.time()-_t0:.1f}s", flush=True)
    _t0 = _time.time()
    TL, TH, in_maps = in_maps
    epsl = tuple(float(v) for v in np.asarray(eps_param, np.float32))
    key = (TL, TH, epsl, _dbg)
    if key not in _cache:
        _cache[key] = _build(TL, TH, epsl, dbg=_dbg)
    nc = _cache[key]
    print(f"[kernel] build {_time.time()-_t0:.1f}s", flush=True)
    _t0 = _time.time()
    res = run_bass_kernel_spmd(nc, in_maps, list(range(NCORES)), trace=_trace)
    print(f"[kernel] run {_time.time()-_t0:.1f}s", flush=True)
    rep = np.concatenate([np.asarray(res.results[c]["out"], np.float32)
                          for c in range(NCORES)], 0)
    x = np.asarray(x, np.float32)
    out = x.copy()
    B, N1, _ = x.shape
    mask = np.asarray(x_mask, np.float32).reshape(V, 1)
    if not np.all(mask == 1.0):
        xn = x[:, 1:, :].reshape(V, D)
        rep = xn + (rep - xn) * mask
    out[:, 1:, :] = rep.reshape(B, N1 - 1, D)
    if _trace or _dbg:
        return out, res
    return out

